# revision 19
# baseline (speedup 1.0000x reference)
"""2-layer GraphSAGE (mean aggregation) on 8 Trainium2 NeuronCores — v2.

Strategy (dst-sharded, transform-first, bf16 tables):
- 100000 nodes padded to 100352 = 8 x 12544 (12500 real per core). Core k
  owns dst nodes [k*12500, (k+1)*12500), processed as NB=98 blocks of 128.
- Transform-first: layer-l messages are y_l = h @ W_l computed BEFORE
  aggregation (mean commutes with the linear map), so gathers move 64-wide
  transformed features instead of 128-wide raw ones. Tables are stored as
  [100352, 128] bf16 rows (256B — the dma_gather minimum elem size; upper
  64 cols are never read). Each core computes its own shard's table tile
  on PE (bf16 matmuls) and the shards are AllGather'ed.
- Edges grouped by (dst block, src chunk of 25088 rows) — 4 chunks keep
  gather indices int16-addressable. Segments padded to 128-position tiles
  (idx 0 + sentinel seg id -> zero contribution).
- The position stream is ordered chunk-major within GROUPs of blocks so a
  single dma_gather call (one source chunk) spans many blocks: far fewer
  calls -> amortizes the ~1us fixed SWDGE descriptor-generation cost.
- Aggregation per 128-position tile: indicator [128 pos, 128 dst] built on
  DVE in bf16 with ONE fused scalar_tensor_tensor per (block, chunk)
  (iota_rep == broadcast seg), then bf16 matmuls accumulate per-dst sums
  in PSUM (1 cycle/row vs 4 for fp32).
- Block epilogue: out = (acc * 1/cnt) + y_r[block] in one fused DVE op
  (y_r = h @ W_r + b, self term, computed per-shard and kept in SBUF);
  ReLU+bf16-cast for layer 1, fp32 write-out for layer 2. No transposes
  or weight matmuls in the inner loop.
"""
import sys
sys.path.insert(0, "/opt/trn_rl_repo")
import numpy as np
import ml_dtypes

import concourse.bass as bass
import concourse.bacc as bacc
import concourse.mybir as mybir
import concourse.tile as tile
from concourse.bass_utils import run_bass_kernel_spmd
from concourse.masks import make_identity

BF16 = ml_dtypes.bfloat16

N_NODES = 100000
N_EDGES = 1600000
F_IN = 128
F_OUT = 64
P = 8                  # cores
NREAL = 12500          # real dsts per core
NL = 13312             # padded dsts per core (= 104 * 128)
BLK = 128              # dsts per block
NB = NL // BLK         # 104 blocks
NG = P * NL            # 106496 padded global rows
CHUNK = 26624          # rows per gather chunk (4 * 26624 = NG, int16-safe)
NCHUNK = NG // CHUNK   # 4
GROUP = 8              # blocks per gather group
NGROUP = NB // GROUP   # 13
SENT = 999.0           # sentinel seg id (exact in bf16, != any iota value)

# gather call shape: None => one call per (group, chunk); else max cols/call
SINGLE_PACKET = False
CALL_COLS = None


def _wrap16(flat_idx):
    """dma_gather index layout: position j -> [j%16, j//16], replicated x8."""
    w = flat_idx.reshape(-1, 16).T.copy()
    return np.tile(w, (8, 1))


def _balance(cnt4):
    """Assign NREAL dsts to NB blocks of <=128 slots, flattening the
    per-(block, chunk) edge-count sums (keeps every segment <= 4 cols)."""
    total = cnt4.sum(axis=1)
    order = np.argsort(-total, kind="stable")
    sums = np.zeros((NB, NCHUNK), dtype=np.int64)
    fill = np.zeros(NB, dtype=np.int64)
    slot_of = np.empty(NREAL, dtype=np.int64)
    for d in order:
        cand = (sums + cnt4[d]).max(axis=1)
        cand[fill >= BLK] = 1 << 40
        b = int(np.argmin(cand))
        slot_of[d] = b * BLK + fill[b]
        fill[b] += 1
        sums[b] += cnt4[d]
    return slot_of


def _preprocess(edge_index):
    """Host-side graph layout (structure only, no feature math).

    Every core's nodes are permuted into "slots" (slot_of) that balance
    per-(dst block, src chunk) edge counts. With CHUNK = 2*NL a src's
    chunk is score//2 regardless of the permutation, so the same slot_of
    consistently orders the output blocks, the self-term columns, AND the
    gather-table rows (h1 blocks write the layer-2 table directly).
    """
    src = np.asarray(edge_index[0], dtype=np.int64)
    dst = np.asarray(edge_index[1], dtype=np.int64)
    dcore = dst // NREAL
    score = src // NREAL
    src_local = src - score * NREAL
    chunk = score // 2                   # == (score*NL + slot) // CHUNK

    # slot assignment per core (depends only on src-core pairs)
    slot_all = []
    for k in range(P):
        sel = dcore == k
        cnt4 = np.zeros((NREAL, NCHUNK), dtype=np.int64)
        np.add.at(cnt4, (dst[sel] - k * NREAL, chunk[sel]), 1)
        slot_all.append(_balance(cnt4))

    # src table row (within chunk) via the src core's permutation
    slot_src = np.empty(len(src), dtype=np.int64)
    for s in range(P):
        sel = score == s
        slot_src[sel] = slot_all[s][src_local[sel]]
    loc = (score % 2) * NL + slot_src    # 0..CHUNK-1, int16-safe

    rcnt_all = []
    counts = np.zeros((P, NB, NCHUNK), dtype=np.int64)
    per_core = []
    for k in range(P):
        sel = dcore == k
        ch = chunk[sel]
        lo = loc[sel]
        slot = slot_all[k][dst[sel] - k * NREAL]
        cnt = np.bincount(slot, minlength=NL).astype(np.float32)
        rcnt = 1.0 / np.maximum(cnt, 1.0)
        rcnt_all.append(rcnt.reshape(NB, BLK).T.copy())  # [128, NB]
        key = (slot // BLK) * NCHUNK + ch
        order = np.argsort(key, kind="stable")
        counts[k] = np.bincount(key, minlength=NB * NCHUNK).reshape(NB, NCHUNK)
        per_core.append((lo[order], (slot % BLK)[order]))
    ncols_u = np.ceil(counts.max(axis=0) / 128).astype(np.int64)  # [NB, NCHUNK]
    ncols_u = np.maximum(ncols_u, 1)

    # position stream: for group: for chunk: for block in group: segment
    # seg_col_off[b][c] = starting column of that segment (global)
    seg_col_off = np.zeros((NB, NCHUNK), dtype=np.int64)
    group_col_off = []           # starting column of each group
    runs = []                    # (group, chunk, global_start_col, ncols)
    col = 0
    for g in range(NGROUP):
        group_col_off.append(col)
        for c in range(NCHUNK):
            run_start = col
            for b in range(g * GROUP, (g + 1) * GROUP):
                seg_col_off[b, c] = col
                col += int(ncols_u[b, c])
            runs.append((g, c, run_start, col - run_start))
    total_cols = col
    group_cols = [group_col_off[g + 1] - group_col_off[g]
                  for g in range(NGROUP - 1)] + [total_cols - group_col_off[-1]]

    # per-core idx / seg streams
    idx_cores = []
    seg_cores = []
    for k in range(P):
        starts_flat = np.concatenate([[0], np.cumsum(counts[k].reshape(-1))])
        lo_o, m_o = per_core[k]
        idx_flat = np.zeros(total_cols * 128, dtype=np.int16)
        seg_flat = np.full(total_cols * 128, SENT, dtype=np.float32)
        for b in range(NB):
            for c in range(NCHUNK):
                i = b * NCHUNK + c
                s, e = starts_flat[i], starts_flat[i + 1]
                n = e - s
                st = seg_col_off[b, c] * 128
                idx_flat[st:st + n] = lo_o[s:e]
                seg_flat[st:st + n] = m_o[s:e]
        idx_cores.append(_wrap16(idx_flat))                    # [128, total_cols*8]
        seg_cores.append(
            seg_flat.reshape(total_cols, 128).T.astype(BF16).copy())  # [128, TC]

    ncmax = int(ncols_u.max())
    return dict(runs=runs, ncols_u=ncols_u, seg_col_off=seg_col_off,
                group_col_off=group_col_off, group_cols=group_cols,
                total_cols=total_cols, ncmax=ncmax, slot_of=slot_all,
                idx_cores=idx_cores, seg_cores=seg_cores, rcnt=rcnt_all)


def _build(meta, rep=1, mode="full", single_packet=None, call_cols=None):
    if single_packet is None:
        single_packet = SINGLE_PACKET
    if call_cols is None:
        call_cols = CALL_COLS
    # split (group, chunk) runs into gather calls
    calls = []
    for (g, c, start, ncols) in meta["runs"]:
        cap = ncols if call_cols is None else call_cols
        done = 0
        while done < ncols:
            piece = min(ncols - done, cap)
            calls.append((c, start + done, piece))
            done += piece
    ncols_u = meta["ncols_u"]
    seg_col_off = meta["seg_col_off"]
    group_col_off = meta["group_col_off"]
    TC = meta["total_cols"]
    GC_max = max(meta["group_cols"])
    ncmax = meta["ncmax"]

    nc = bacc.Bacc("TRN2", target_bir_lowering=False, debug=False,
                   num_devices=P, num_swdge_queues=4)
    dt = mybir.dt
    xT_d = nc.dram_tensor("xT", [F_IN, NL], dt.bfloat16, kind="ExternalInput")
    idx_d = nc.dram_tensor("idx", [128, TC * 8], dt.int16, kind="ExternalInput")
    seg_d = nc.dram_tensor("seg", [128, TC], dt.bfloat16, kind="ExternalInput")
    rcnt_d = nc.dram_tensor("rcnt", [128, NB], dt.float32, kind="ExternalInput")
    iota_d = nc.dram_tensor("iota", [128, ncmax * 128], dt.bfloat16,
                            kind="ExternalInput")
    wl1_d = nc.dram_tensor("W_l1", [F_IN, F_OUT], dt.bfloat16, kind="ExternalInput")
    wr1_d = nc.dram_tensor("W_r1", [F_IN, F_OUT], dt.bfloat16, kind="ExternalInput")
    b1_d = nc.dram_tensor("b1", [1, F_OUT], dt.bfloat16, kind="ExternalInput")
    wl2_d = nc.dram_tensor("W_l2", [F_OUT, F_OUT], dt.bfloat16, kind="ExternalInput")
    wr2_d = nc.dram_tensor("W_r2", [F_OUT, F_OUT], dt.bfloat16, kind="ExternalInput")
    b2_d = nc.dram_tensor("b2", [1, F_OUT], dt.bfloat16, kind="ExternalInput")
    out_d = nc.dram_tensor("out", [NL, F_OUT], dt.float32, kind="ExternalOutput")

    ytab1_shard = nc.dram_tensor("ytab1_shard", [NL, F_IN], dt.bfloat16)
    ytab1 = nc.dram_tensor("ytab1", [NG, F_IN], dt.bfloat16, addr_space="Shared")
    ytab2_shard = nc.dram_tensor("ytab2_shard", [NL, F_IN], dt.bfloat16)
    ytab2 = nc.dram_tensor("ytab2", [NG, F_IN], dt.bfloat16, addr_space="Shared")

    ident = mybir.ActivationFunctionType
    alu = mybir.AluOpType

    with tile.TileContext(nc) as tc:
        with (
            tc.tile_pool(name="const", bufs=1) as constp,
            tc.tile_pool(name="persist", bufs=1) as persistp,
            tc.tile_pool(name="xp", bufs=3) as xp,
            tc.tile_pool(name="stagep", bufs=2) as stagep,
            tc.tile_pool(name="indp", bufs=4) as indp,
            tc.tile_pool(name="op", bufs=4) as op,
            tc.tile_pool(name="ps_acc", bufs=2, space="PSUM") as ps_acc,
            tc.tile_pool(name="ps_y", bufs=2, space="PSUM") as ps_y,
            tc.tile_pool(name="ps_t", bufs=2, space="PSUM") as ps_t,
        ):
            idx_t = constp.tile([128, TC * 8], dt.int16)
            nc.sync.dma_start(idx_t[:], idx_d[:])
            seg_t = constp.tile([128, TC], dt.bfloat16)
            nc.sync.dma_start(seg_t[:], seg_d[:])
            rcnt_t = constp.tile([128, NB], dt.float32)
            nc.sync.dma_start(rcnt_t[:], rcnt_d[:])
            iota_t = constp.tile([128, ncmax * 128], dt.bfloat16)
            nc.sync.dma_start(iota_t[:], iota_d[:])
            wl1_t = constp.tile([F_IN, F_OUT], dt.bfloat16)
            nc.sync.dma_start(wl1_t[:], wl1_d[:])
            wr1_t = constp.tile([F_IN, F_OUT], dt.bfloat16)
            nc.sync.dma_start(wr1_t[:], wr1_d[:])
            wl2_t = constp.tile([F_OUT, F_OUT], dt.bfloat16)
            nc.sync.dma_start(wl2_t[:], wl2_d[:])
            wr2_t = constp.tile([F_OUT, F_OUT], dt.bfloat16)
            nc.sync.dma_start(wr2_t[:], wr2_d[:])
            b1_t = constp.tile([1, F_OUT], dt.bfloat16)
            nc.sync.dma_start(b1_t[:], b1_d[:])
            b2_t = constp.tile([1, F_OUT], dt.bfloat16)
            nc.sync.dma_start(b2_t[:], b2_d[:])
            ones_t = constp.tile([1, 128], dt.bfloat16)
            nc.vector.memset(ones_t[:], 1.0)
            id_t = constp.tile([128, 128], dt.bfloat16)
            make_identity(nc, id_t[:])
            # self-term buffer: y1r during L1, overwritten by y2r per block
            yr_t = persistp.tile([128, NB * F_OUT], dt.float32)

            qn = [0]
            last_junk = [None]

            def gather_group(g, table, stage):
                c0 = group_col_off[g]
                for (c, start_col, piece) in calls:
                    if not (c0 <= start_col < c0 + meta["group_cols"][g]):
                        continue
                    w0 = start_col - c0
                    nc.gpsimd.dma_gather(
                        out_ap=stage[:, w0 * F_IN:(w0 + piece) * F_IN]
                            .rearrange("p (c f) -> p c f", f=F_IN),
                        in_ap=table[c * CHUNK:(c + 1) * CHUNK, :],
                        idxs_ap=idx_t[:, start_col * 8:(start_col + piece) * 8],
                        num_idxs=piece * 128, num_idxs_reg=piece * 128,
                        elem_size=F_IN, single_packet=single_packet,
                        queue_num=qn[0] % 4)
                    qn[0] += 1
                    if mode == "gather":
                        junk = op.tile([128, F_OUT], dt.float32, tag="junk")
                        nc.vector.tensor_copy(
                            junk[:], stage[:, w0 * F_IN:w0 * F_IN + F_OUT])
                        last_junk[0] = junk

            def aggregate_block(b, g, stage):
                """acc[128 dst, 64] = sum of staged messages for block b."""
                c0 = group_col_off[g]
                acc = ps_acc.tile([BLK, F_OUT], dt.float32, tag="acc")
                nsegs = [int(ncols_u[b, c]) for c in range(NCHUNK)]
                total = sum(nsegs)
                done = 0
                for c in range(NCHUNK):
                    ncol = nsegs[c]
                    if ncol == 0:
                        continue
                    s0 = seg_col_off[b, c]
                    w0 = s0 - c0
                    ind = indp.tile([128, ncmax * 128], dt.bfloat16, tag="ind")
                    nc.vector.scalar_tensor_tensor(
                        out=ind[:, :ncol * 128]
                            .rearrange("p (c f) -> p c f", f=128),
                        in0=iota_t[:, :ncol * 128]
                            .rearrange("p (c f) -> p c f", f=128),
                        scalar=0.0,
                        in1=seg_t[:, s0:s0 + ncol].unsqueeze(2)
                            .broadcast_to([128, ncol, 128]),
                        op0=alu.add, op1=alu.is_equal)
                    for j in range(ncol):
                        nc.tensor.matmul(
                            acc[:],
                            lhsT=ind[:, j * 128:(j + 1) * 128],
                            rhs=stage[:, (w0 + j) * F_IN:(w0 + j) * F_IN + F_OUT],
                            start=(done == 0), stop=(done == total - 1))
                        done += 1
                return acc

            def shard_table_tile(b, lhsT_ap, wl, wr, bias, tab_dram):
                """y_l tile -> bf16 table rows; y_r tile (+bias) -> yr_t."""
                psl = ps_y.tile([128, F_OUT], dt.float32, tag="psl")
                nc.tensor.matmul(psl[:], lhsT=lhsT_ap, rhs=wl[:],
                                 start=True, stop=True)
                ya = op.tile([128, F_OUT], dt.bfloat16, tag="ya")
                nc.scalar.activation(out=ya[:], in_=psl[:], func=ident.Copy)
                nc.sync.dma_start(tab_dram[b * 128:(b + 1) * 128, :F_OUT], ya[:])
                psr = ps_y.tile([128, F_OUT], dt.float32, tag="psr")
                nc.tensor.matmul(psr[:], lhsT=lhsT_ap, rhs=wr[:],
                                 start=True, stop=False)
                nc.tensor.matmul(psr[:], lhsT=ones_t[:1, :128], rhs=bias[:],
                                 start=False, stop=True)
                nc.vector.tensor_copy(yr_t[:, b * F_OUT:(b + 1) * F_OUT], psr[:])

            for _r in range(rep):
                # ---- phase A: y1 tables from x shard ----
                for b in range(NB):
                    xt = xp.tile([F_IN, 128], dt.bfloat16, tag="xt")
                    nc.sync.dma_start(xt[:], xT_d[:, b * 128:(b + 1) * 128])
                    shard_table_tile(b, xt[:], wl1_t, wr1_t, b1_t, ytab1_shard)
                if mode not in ("gather", "noag"):
                    nc.gpsimd.collective_compute(
                        "AllGather", alu.bypass,
                        replica_groups=[list(range(P))],
                        ins=[ytab1_shard[:]], outs=[ytab1[:]])

                # ---- layer 1 aggregation + h1/y2 prep ----
                for g in range(NGROUP):
                    stage = stagep.tile([128, GC_max * F_IN], dt.bfloat16,
                                        tag="stage")
                    if mode != "compute":
                        gather_group(g, ytab1, stage)
                    else:
                        nc.vector.memset(stage[:, :128], 0.0)
                    if mode == "gather":
                        continue
                    for b in range(g * GROUP, (g + 1) * GROUP):
                        acc = aggregate_block(b, g, stage)
                        t1 = op.tile([128, F_OUT], dt.float32, tag="t1")
                        nc.vector.scalar_tensor_tensor(
                            out=t1[:], in0=acc[:], scalar=rcnt_t[:, b:b + 1],
                            in1=yr_t[:, b * F_OUT:(b + 1) * F_OUT],
                            op0=alu.mult, op1=alu.add)
                        h1b = op.tile([128, F_OUT], dt.bfloat16, tag="h1b")
                        nc.vector.tensor_scalar_max(h1b[:], t1[:], 0.0)
                        # transpose h1 block for the y2 matmuls
                        tp = ps_t.tile([F_OUT, 128], dt.bfloat16, tag="tp")
                        nc.tensor.transpose(out=tp[:], in_=h1b[:],
                                            identity=id_t[:])
                        h1t = op.tile([F_OUT, 128], dt.bfloat16, tag="h1t")
                        nc.scalar.activation(out=h1t[:], in_=tp[:],
                                             func=ident.Copy)
                        shard_table_tile(b, h1t[:], wl2_t, wr2_t, b2_t,
                                         ytab2_shard)
                if mode not in ("gather", "noag"):
                    nc.gpsimd.collective_compute(
                        "AllGather", alu.bypass,
                        replica_groups=[list(range(P))],
                        ins=[ytab2_shard[:]], outs=[ytab2[:]])

                # ---- layer 2 aggregation ----
                for g in range(NGROUP):
                    stage = stagep.tile([128, GC_max * F_IN], dt.bfloat16,
                                        tag="stage")
                    if mode != "compute":
                        gather_group(g, ytab2, stage)
                    else:
                        nc.vector.memset(stage[:, :128], 0.0)
                    if mode == "gather":
                        continue
                    for b in range(g * GROUP, (g + 1) * GROUP):
                        acc = aggregate_block(b, g, stage)
                        ot = op.tile([128, F_OUT], dt.float32, tag="ot")
                        nc.vector.scalar_tensor_tensor(
                            out=ot[:], in0=acc[:], scalar=rcnt_t[:, b:b + 1],
                            in1=yr_t[:, b * F_OUT:(b + 1) * F_OUT],
                            op0=alu.mult, op1=alu.add)
                        nc.sync.dma_start(out_d[b * 128:(b + 1) * 128, :], ot[:])
                if mode == "gather":
                    nc.sync.dma_start(out_d[0:128, :], last_junk[0][:])

    nc.finalize()
    return nc


def make_in_maps(x, W_l1, W_r1, b1, W_l2, W_r2, b2, meta):
    x = np.asarray(x, dtype=np.float32)
    xT_all = np.zeros((P, F_IN, NL), dtype=BF16)
    for k in range(P):
        xs = np.zeros((NL, F_IN), np.float32)
        xs[meta["slot_of"][k]] = x[k * NREAL:(k + 1) * NREAL]
        xT_all[k] = xs.T.astype(BF16)
    ncmax = meta["ncmax"]
    iota = np.tile(np.arange(128, dtype=np.float32), ncmax)
    iota = np.broadcast_to(iota.astype(BF16), (128, ncmax * 128)).copy()
    in_maps = []
    for k in range(P):
        in_maps.append({
            "xT": xT_all[k],
            "idx": meta["idx_cores"][k],
            "seg": meta["seg_cores"][k],
            "rcnt": meta["rcnt"][k],
            "iota": iota,
            "W_l1": np.asarray(W_l1, np.float32).astype(BF16),
            "W_r1": np.asarray(W_r1, np.float32).astype(BF16),
            "b1": np.asarray(b1, np.float32).reshape(1, F_OUT).astype(BF16),
            "W_l2": np.asarray(W_l2, np.float32).astype(BF16),
            "W_r2": np.asarray(W_r2, np.float32).astype(BF16),
            "b2": np.asarray(b2, np.float32).reshape(1, F_OUT).astype(BF16),
        })
    return in_maps


def kernel(x, edge_index, W_l1, W_r1, b1, W_l2, W_r2, b2):
    meta = _preprocess(np.asarray(edge_index))
    in_maps = make_in_maps(x, W_l1, W_r1, b1, W_l2, W_r2, b2, meta)
    nc = _build(meta)
    res = run_bass_kernel_spmd(nc, in_maps, core_ids=list(range(P)))
    out = np.concatenate(
        [res.results[k]["out"][meta["slot_of"][k]] for k in range(P)], axis=0)
    return out.astype(np.float32)


if __name__ == "__main__":
    rng = np.random.default_rng(0)
    x = rng.normal(size=(N_NODES, F_IN)).astype(np.float32)
    ei = rng.integers(0, N_NODES, size=(2, N_EDGES)).astype(np.int64)
    wl1 = rng.normal(size=(F_IN, F_OUT)).astype(np.float32) / np.sqrt(F_IN)
    wr1 = rng.normal(size=(F_IN, F_OUT)).astype(np.float32) / np.sqrt(F_IN)
    wl2 = rng.normal(size=(F_OUT, F_OUT)).astype(np.float32) / np.sqrt(F_OUT)
    wr2 = rng.normal(size=(F_OUT, F_OUT)).astype(np.float32) / np.sqrt(F_OUT)
    b1 = np.zeros(F_OUT, np.float32)
    b2 = np.zeros(F_OUT, np.float32)
    out = kernel(x, ei, wl1, wr1, b1, wl2, wr2, b2)
    print("out", out.shape, out.dtype, float(np.abs(out).mean()))
